# revision 4
# baseline (speedup 1.0000x reference)
"""Trainium2 Bass kernel for nn_MeshTransformer — fp16 pipeline.

out[b,s] = sum_p w[b,s,p] * (scale[b,s] * (verts @ R[b,s,p]^T) + t[b,s,p])
collapses per slot to  out[b,s] = verts_h @ A[b,s]  with A = [scale*Rbar^T; tbar]
(4x3), Rbar = sum_p w_p R_p, tbar = sum_p w_p t_p.

Correctness gate is rel_err < 2e-2 (norm), so the whole pipeline runs in
fp16 where it buys bandwidth (measured end-to-end rel err ~3e-4):
  - inputs packed fp16 [slots, 449] = [transforms(384) | w(64) | scale(1)]
  - PE matmul in fp16 with an interleaved rhs table so PSUM comes out
    already in (v*3+i) output order -> PSUM->SBUF copies are contiguous
  - output stored fp16 (halves the dominant HBM write: 63MB -> 31.5MB),
    upcast to f32 on the host during the gather step

Per-core structure (data-parallel over B, 8 b's per core = 256 slots,
two 128-slot partition tiles):
  1. one DMA per slot tile loads the packed fp16 inputs
  2. DVE add_range_wrap wraps angles into [-pi,pi] (one op per sin/cos
     half), written c-major so one contiguous ACT Sin yields the six
     factor arrays sa..cc as contiguous 64-wide blocks
  3. GPSIMD forms the 5 weighted trig products; DVE 16 affine_mul_reduce
     over P=64 build the 12 A entries per slot (per-slot +/-scale folded
     into the AMR scale slot); 4 32x32 DVE transposes -> lhsT [12,128],
     replicated to bases 32/64/96 via DVE+GPSIMD
  5. PE: 16 matmuls per tile, K=12, N=512 (one PSUM bank each), rhs is a
     host-built [12, 7686] fp16 table Rh[4i+k, 3v+i] = verts_h[k,v] so the
     matmul directly emits interleaved (v,i); row groups round-robin
  6. PSUM->SBUF cast-copies (f32 -> fp16) in 4-bank blocks, split ACT/DVE
  7. per-block DMA stores stream each [128, 7686] fp16 tile to DRAM
"""

import sys

if "/opt/trn_rl_repo" not in sys.path:
    sys.path.insert(0, "/opt/trn_rl_repo")

import numpy as np

import concourse.bacc as bacc
import concourse.mybir as mybir
import concourse.tile as tile
from concourse.bass_utils import run_bass_kernel_spmd

F32 = mybir.dt.float32
F16 = mybir.dt.float16
ALU = mybir.AluOpType
ACTF = mybir.ActivationFunctionType

B, S, P, V = 64, 32, 64, 2562
V3 = V * 3                  # 7686 output cols per slot
NCORES = 8
BL = B // NCORES            # batches per core
SLOTS = BL * S              # 256 slots per core
PT = 128                    # slots per partition-tile
NT = SLOTS // PT            # 2 slot tiles
PI = float(np.pi)
INP_W = P * 6 + P + 1       # packed [transforms(384) | w(64) | scale(1)] = 449
BLKW = 2048                 # copy/store block width (4 PSUM banks)
NBLK = 4                    # blocks per tile: 3 x 2048 + 1 x 1542
DVE_BLOCKS = (1,)           # blocks whose PSUM->SBUF copy runs on DVE (rest ACT)


def _blk_cols(b):
    lo = BLKW * b
    hi = min(BLKW * (b + 1), V3)
    return lo, hi


def _views(inp_t):
    tr3 = inp_t[:, 0:384].rearrange("p (q c) -> p q c", c=6)   # [128,64,6]
    ang = tr3[:, :, 3:6]
    w = inp_t[:, 384:448]
    scl = inp_t[:, 448:449]
    return tr3, ang, w, scl


def _prep_a(nc, pools, inp_t):
    """Stage A: range-wrap + Sin + the per-slot scale columns."""
    io, wk, scr, pp = pools
    tr3, ang, w, scl = _views(inp_t)

    # wrapped angles, written c-major so one contiguous Sin covers both halves
    u = wk.tile([PT, 384], F16, tag="u")
    u_s = u[:, 0:192].rearrange("p (c q) -> p q c", q=P)
    u_c = u[:, 192:384].rearrange("p (c q) -> p q c", q=P)
    nc.vector.add_range_wrap(u_s, ang, 0.0, PI, 2.0 * PI)
    nc.vector.add_range_wrap(u_c, ang, 0.5 * PI, PI, 2.0 * PI)

    # sincos[:, 64*k : 64*(k+1)] = contiguous factor arrays sa sb sc ca cb cc
    sincos = wk.tile([PT, 384], F16, tag="sincos")
    nc.scalar.activation(sincos[:], u[:], ACTF.Sin)

    # +/-scale as f32 [P,1] — fed to the AMRs as their per-partition scale
    # so the rotation entries come out pre-multiplied (translations use 1.0)
    scl2 = wk.tile([PT, 2], F32, tag="scl2")
    nc.vector.tensor_copy(scl2[:, 0:1], scl)
    nc.vector.tensor_scalar_mul(scl2[:, 1:2], scl2[:, 0:1], -1.0)
    return sincos, scl2


def _prep_b(nc, pools, inp_t, sincos, scl2):
    """Stage B: products + 16 AMR reductions + transpose -> lhsT at[108, 128]
    fp16 with L[32g + 4i + j, slot] = A[slot][j, i] at 4 row-group bases."""
    io, wk, scr, pp = pools
    tr3, ang, w, scl = _views(inp_t)
    f6 = sincos[:].rearrange("p (k q) -> p k q", q=P)
    sa, sb, sc_, ca, cb, cc = (f6[:, k, :] for k in range(6))

    prod = wk.tile([PT, 5 * P], F16, tag="prod")
    p5 = prod[:].rearrange("p (k q) -> p k q", q=P)
    wcb, wca, wsa, wsasb, wcasb = (p5[:, k, :] for k in range(5))
    pscl, nscl = scl2[:, 0:1], scl2[:, 1:2]

    # acol[slot, 4i + j]: j=0..2 -> Rbar[i][j], j=3 -> tbar[i]
    # XYZ euler: R00=cb*cc R01=-cb*sc R02=sb ; R10=ca*sc+sa*sb*cc
    # R11=ca*cc-sa*sb*sc R12=-sa*cb ; R20=sa*sc-ca*sb*cc R21=sa*cc+ca*sb*sc
    # R22=ca*cb
    acol = wk.tile([PT, 12], F32, tag="acol")
    acolp = wk.tile([PT, 4], F32, tag="acolp")

    def red(dst, col, in0, in1, scale=1.0):
        out_scr = scr.tile([PT, P], F32, tag="amr_scr")
        nc.vector.affine_mul_reduce(
            out=out_scr[:], accum_out=dst[:, col:col + 1],
            in0=in0, in1=in1, scale=scale, bias=0.0)

    tx, ty, tz = tr3[:, :, 0], tr3[:, :, 1], tr3[:, :, 2]

    # the t-entries depend only on the input tile, so they go first — DVE
    # runs them while ACT still produces sincos; then the 5 products; the
    # rotation entries carry the per-slot +/-scale through the AMR scale slot
    red(acol, 3, w, tx)                        # t0
    red(acol, 7, w, ty)                        # t1
    red(acol, 11, w, tz)                       # t2
    nc.gpsimd.tensor_mul(wcb, w, cb)
    nc.gpsimd.tensor_mul(wca, w, ca)
    nc.gpsimd.tensor_mul(wsa, w, sa)
    nc.gpsimd.tensor_mul(wsasb, wsa, sb)
    nc.gpsimd.tensor_mul(wcasb, wca, sb)
    red(acol, 2, w, sb, scale=pscl)            # M02
    red(acol, 0, wcb, cc, scale=pscl)          # M00
    red(acol, 1, wcb, sc_, scale=nscl)         # M01
    red(acol, 4, wca, sc_, scale=pscl)         # M10 part 1
    red(acolp, 0, wsasb, cc, scale=pscl)       # M10 part 2
    red(acol, 5, wca, cc, scale=pscl)          # M11 part 1
    red(acolp, 1, wsasb, sc_, scale=nscl)      # M11 part 2
    red(acol, 6, wsa, cb, scale=nscl)          # M12
    red(acol, 8, wsa, sc_, scale=pscl)         # M20 part 1
    red(acolp, 2, wcasb, cc, scale=nscl)       # M20 part 2
    red(acol, 9, wsa, cc, scale=pscl)          # M21 part 1
    red(acolp, 3, wcasb, sc_, scale=pscl)      # M21 part 2
    red(acol, 10, wca, cb, scale=pscl)         # M22

    # merge two-part sums: acol cols {4,5,8,9} += acolp cols {0,1,2,3}
    am = acol[:].rearrange("p (i j) -> p i j", j=4)
    nc.vector.tensor_tensor(am[:, 1:3, 0:2], am[:, 1:3, 0:2],
                            acolp[:].rearrange("p (i j) -> p i j", j=2),
                            ALU.add)

    acolh = wk.tile([PT, 32], F16, tag="acolh")
    nc.vector.tensor_copy(acolh[:, 0:12], acol[:])

    at = wk.tile([108, PT], F16, tag="at")
    for b in range(4):
        nc.vector.transpose(at[0:32, 32 * b:32 * b + 32],
                            acolh[32 * b:32 * b + 32, 0:32])
    # replicate to the other 3 row-group bases; GPSIMD+DVE keep the busy
    # ACT engine out of this small matmul-ready chain
    nc.vector.tensor_copy(at[32:44, :], at[0:12, :])
    nc.gpsimd.tensor_copy(at[64:76, :], at[0:12, :])
    nc.gpsimd.tensor_copy(at[96:108, :], at[0:12, :])
    return at


def _phase_mm(nc, t, pools, at, vt_rep, out_d):
    io, wk, scr, pp = pools
    out_t = io.tile([PT, V3], F16, tag="out")
    for bk in range(NBLK):
        lo, hi = _blk_cols(bk)
        ps = pp.tile([PT, BLKW], F32, tag="ps")
        for j in range((hi - lo + 511) // 512):
            clo = lo + 512 * j
            chi = min(clo + 512, V3)
            g = (clo // 512) % 4
            nc.tensor.matmul(ps[:, 512 * j:512 * j + (chi - clo)],
                             at[32 * g:32 * g + 12, :],
                             vt_rep[32 * g:32 * g + 12, clo:chi],
                             start=True, stop=True, tile_position=(32 * g, 0))
        if bk in DVE_BLOCKS:
            nc.vector.tensor_copy(out_t[:, lo:hi], ps[:, 0:hi - lo])
            nc.sync.dma_start(out_d[t * PT:(t + 1) * PT, lo:hi],
                              out_t[:, lo:hi])
        else:
            nc.scalar.copy(out_t[:, lo:hi], ps[:, 0:hi - lo])
            nc.sync.dma_start(out_d[t * PT:(t + 1) * PT, lo:hi],
                              out_t[:, lo:hi])


def build(loop_iters: int = 0, sim_safe: bool = False,
          bench_internal_out: bool = False, unroll: int = 0,
          barrier_between: bool = False, loop_unroll: int = 1):
    """Build + compile the per-core program. loop_iters=0 -> straight-line
    single pass (grading); loop_iters=N -> For_i loop whose body runs
    loop_unroll back-to-back passes (the passes pipeline across engines;
    For_i's all-engine barrier only fires once per iteration) for
    wall-clock timing. sim_safe accepted for API compat (unused)."""
    nc = bacc.Bacc("TRN2", target_bir_lowering=False, debug=False)
    vt_d = nc.dram_tensor("vt", [48, V3], F16, kind="ExternalInput")
    inp_d = nc.dram_tensor("inp", [SLOTS, INP_W], F16, kind="ExternalInput")
    if bench_internal_out:
        # timing builds write to internal DRAM (identical HBM traffic) and
        # expose only a tiny dummy output, so host<->device transfer noise
        # doesn't pollute wall-clock differencing.
        out_d = nc.dram_tensor("outbuf", [SLOTS, V3], F16)
        dummy_d = nc.dram_tensor("out", [1, 16], F32, kind="ExternalOutput")
    else:
        out_d = nc.dram_tensor("out", [SLOTS, V3], F16, kind="ExternalOutput")
        dummy_d = None

    with tile.TileContext(nc) as tc:
        with (
            tc.tile_pool(name="const", bufs=1) as cpool,
            tc.tile_pool(name="io", bufs=3) as io,
            tc.tile_pool(name="wk", bufs=3) as wk,
            tc.tile_pool(name="scr", bufs=6) as scr,
            tc.tile_pool(name="psum", bufs=2, space="PSUM") as pp,
        ):
            # interleaved verts_h^T table replicated at the 4 row-group bases;
            # loaded on the ACT HWDGE ring so it never queues ahead of the
            # latency-critical input loads (first use is the first matmul)
            vt_rep = cpool.tile([108, V3], F16)
            for g in range(4):
                nc.gpsimd.dma_start(vt_rep[32 * g:32 * g + 12, :],
                                    vt_d[12 * g:12 * g + 12, :])
            pools = (io, wk, scr, pp)

            def passes():
                inps, pa, ats = [], [], []
                for t in range(NT):
                    inp_t = io.tile([PT, INP_W], F16, tag="inp")
                    nc.sync.dma_start(inp_t[:], inp_d[t * PT:(t + 1) * PT, :])
                    inps.append(inp_t)
                for t in range(NT):
                    pa.append(_prep_a(nc, pools, inps[t]))
                for t in range(NT):
                    ats.append(_prep_b(nc, pools, inps[t], *pa[t]))
                for t in range(NT):
                    _phase_mm(nc, t, pools, ats[t], vt_rep, out_d)

            if loop_iters:
                with tc.For_i(0, loop_iters, 1):
                    for _ in range(loop_unroll):
                        passes()
            elif unroll:
                # straight-line repetition for TimelineSim measurement
                # (For_i needs register state the sim lacks); barriers
                # between passes mimic For_i's per-iteration
                # InstAllEngineBarrier
                for i in range(unroll):
                    if i and barrier_between:
                        nc.all_engine_barrier()
                    passes()
            else:
                passes()
            if dummy_d is not None:
                dtile = cpool.tile([1, 16], F32)
                nc.vector.memset(dtile[:], 1.0)
                nc.sync.dma_start(dummy_d[:], dtile[:])

    nc.compile()
    return nc


def _shard_inputs(verts, scales, transforms, prototype_weights):
    verts = np.ascontiguousarray(verts, dtype=np.float32)
    vh = np.concatenate([verts.T, np.ones((1, V), np.float32)],
                        axis=0).astype(np.float16)          # [4, V]
    vt12 = np.zeros((12, V3), np.float16)
    for i in range(3):
        vt12[4 * i:4 * i + 4, i::3] = vh
    vt48 = np.ascontiguousarray(np.vstack([vt12] * 4))      # [48, 7686]

    tr = transforms.reshape(B * S, P * 6).astype(np.float16)
    w = prototype_weights.reshape(B * S, P).astype(np.float16)
    sc = scales.reshape(B * S, 1).astype(np.float16)
    packed = np.concatenate([tr, w, sc], axis=1)            # [2048, 449]

    in_maps = []
    for k in range(NCORES):
        sl = slice(k * SLOTS, (k + 1) * SLOTS)
        in_maps.append({"vt": vt48, "inp": np.ascontiguousarray(packed[sl])})
    return in_maps


_cached_nc = None


def kernel(verts, scales, transforms, prototype_weights):
    global _cached_nc
    verts = np.asarray(verts, dtype=np.float32)
    scales = np.asarray(scales, dtype=np.float32)
    transforms = np.asarray(transforms, dtype=np.float32)
    prototype_weights = np.asarray(prototype_weights, dtype=np.float32)
    if _cached_nc is None:
        _cached_nc = build(loop_iters=0)
    in_maps = _shard_inputs(verts, scales, transforms, prototype_weights)
    res = run_bass_kernel_spmd(_cached_nc, in_maps, core_ids=list(range(NCORES)))
    parts = [np.asarray(res.results[k]["out"]).astype(np.float32)
             .reshape(SLOTS, V, 3) for k in range(NCORES)]
    return np.concatenate(parts, axis=0)


# revision 6
# speedup vs baseline: 1.0022x; 1.0022x over previous
"""Trainium2 Bass kernel for nn_MeshTransformer — fp16 pipeline.

out[b,s] = sum_p w[b,s,p] * (scale[b,s] * (verts @ R[b,s,p]^T) + t[b,s,p])
collapses per slot to  out[b,s] = verts_h @ A[b,s]  with A = [scale*Rbar^T; tbar]
(4x3), Rbar = sum_p w_p R_p, tbar = sum_p w_p t_p.

Correctness gate is rel_err < 2e-2 (norm), so the whole pipeline runs in
fp16 where it buys bandwidth (measured end-to-end rel err ~3e-4):
  - inputs packed fp16 [slots, 449] = [transforms(384) | w(64) | scale(1)]
  - PE matmul in fp16 with an interleaved rhs table so PSUM comes out
    already in (v*3+i) output order -> PSUM->SBUF copies are contiguous
  - output stored fp16 (halves the dominant HBM write: 63MB -> 31.5MB),
    upcast to f32 on the host during the gather step

Per-core structure (data-parallel over B, 8 b's per core = 256 slots,
two 128-slot partition tiles):
  1. one DMA per slot tile loads the packed fp16 inputs
  2. DVE add_range_wrap wraps angles into [-pi,pi] (one op per sin/cos
     half), written c-major so one contiguous ACT Sin yields the six
     factor arrays sa..cc as contiguous 64-wide blocks
  3. GPSIMD forms the 5 weighted trig products; DVE 16 affine_mul_reduce
     over P=64 build the 12 A entries per slot (per-slot +/-scale folded
     into the AMR scale slot); 4 32x32 DVE transposes -> lhsT [12,128],
     replicated to bases 32/64/96 via DVE+GPSIMD
  4. PE: 16 matmuls per tile, K=12, N=512 (one PSUM bank each), rhs is a
     host-built [12, 7686] fp16 table Rh[4i+k, 3v+i] = verts_h[k,v] so the
     matmul directly emits interleaved (v,i); row groups round-robin
  5. PSUM->SBUF cast-copies (f32 -> fp16) in 4-bank blocks, split ACT/DVE
  6. per-block DMA stores stream each [128, 7686] fp16 tile to DRAM
"""

import sys

if "/opt/trn_rl_repo" not in sys.path:
    sys.path.insert(0, "/opt/trn_rl_repo")

import numpy as np

import concourse.bacc as bacc
import concourse.mybir as mybir
import concourse.tile as tile
from concourse.bass_utils import run_bass_kernel_spmd

F32 = mybir.dt.float32
F16 = mybir.dt.float16
ALU = mybir.AluOpType
ACTF = mybir.ActivationFunctionType

B, S, P, V = 64, 32, 64, 2562
V3 = V * 3                  # 7686 output cols per slot
NCORES = 8
BL = B // NCORES            # batches per core
SLOTS = BL * S              # 256 slots per core
PT = 128                    # slots per partition-tile
NT = SLOTS // PT            # 2 slot tiles
PI = float(np.pi)
INP_W = P * 6 + P + 1       # packed [transforms(384) | w(64) | scale(1)] = 449
BLKW = 2048                 # copy/store block width (4 PSUM banks)
NBLK = 4                    # blocks per tile: 3 x 2048 + 1 x 1542
DVE_BLOCKS = (1,)           # blocks whose PSUM->SBUF copy runs on DVE (rest ACT)


def _blk_cols(b):
    lo = BLKW * b
    hi = min(BLKW * (b + 1), V3)
    return lo, hi


def _views(inp_t):
    tr3 = inp_t[:, 0:384].rearrange("p (q c) -> p q c", c=6)   # [128,64,6]
    ang = tr3[:, :, 3:6]
    w = inp_t[:, 384:448]
    scl = inp_t[:, 448:449]
    return tr3, ang, w, scl


def _prep_a(nc, pools, inp_t):
    """Stage A: range-wrap + Sin + the per-slot scale columns."""
    io, wk, scr, pp = pools
    tr3, ang, w, scl = _views(inp_t)

    # wrapped angles, written c-major so one contiguous Sin covers both halves
    u = wk.tile([PT, 384], F16, tag="u")
    u_s = u[:, 0:192].rearrange("p (c q) -> p q c", q=P)
    u_c = u[:, 192:384].rearrange("p (c q) -> p q c", q=P)
    nc.vector.add_range_wrap(u_s, ang, 0.0, PI, 2.0 * PI)
    nc.vector.add_range_wrap(u_c, ang, 0.5 * PI, PI, 2.0 * PI)

    # sincos[:, 64*k : 64*(k+1)] = contiguous factor arrays sa sb sc ca cb cc
    sincos = wk.tile([PT, 384], F16, tag="sincos")
    nc.scalar.activation(sincos[:], u[:], ACTF.Sin)

    # +/-scale as f32 [P,1] — fed to the AMRs as their per-partition scale
    # so the rotation entries come out pre-multiplied (translations use 1.0)
    scl2 = wk.tile([PT, 2], F32, tag="scl2")
    nc.vector.tensor_copy(scl2[:, 0:1], scl)
    nc.vector.tensor_scalar_mul(scl2[:, 1:2], scl2[:, 0:1], -1.0)
    return sincos, scl2


def _prep_b(nc, pools, inp_t, sincos, scl2):
    """Stage B: products + 16 AMR reductions + transpose -> lhsT at[108, 128]
    fp16 with L[32g + 4i + j, slot] = A[slot][j, i] at 4 row-group bases."""
    io, wk, scr, pp = pools
    tr3, ang, w, scl = _views(inp_t)
    f6 = sincos[:].rearrange("p (k q) -> p k q", q=P)
    sa, sb, sc_, ca, cb, cc = (f6[:, k, :] for k in range(6))

    prod = wk.tile([PT, 5 * P], F16, tag="prod")
    p5 = prod[:].rearrange("p (k q) -> p k q", q=P)
    wcb, wca, wsa, wsasb, wcasb = (p5[:, k, :] for k in range(5))
    pscl, nscl = scl2[:, 0:1], scl2[:, 1:2]

    # acol[slot, 4i + j]: j=0..2 -> Rbar[i][j], j=3 -> tbar[i]
    # XYZ euler: R00=cb*cc R01=-cb*sc R02=sb ; R10=ca*sc+sa*sb*cc
    # R11=ca*cc-sa*sb*sc R12=-sa*cb ; R20=sa*sc-ca*sb*cc R21=sa*cc+ca*sb*sc
    # R22=ca*cb
    acol = wk.tile([PT, 12], F32, tag="acol")
    acolp = wk.tile([PT, 4], F32, tag="acolp")

    def red(dst, col, in0, in1, scale=1.0):
        out_scr = scr.tile([PT, P], F32, tag="amr_scr")
        nc.vector.affine_mul_reduce(
            out=out_scr[:], accum_out=dst[:, col:col + 1],
            in0=in0, in1=in1, scale=scale, bias=0.0)

    tx, ty, tz = tr3[:, :, 0], tr3[:, :, 1], tr3[:, :, 2]

    # the t-entries depend only on the input tile, so they go first — DVE
    # runs them while ACT still produces sincos; then the 5 products; the
    # rotation entries carry the per-slot +/-scale through the AMR scale slot
    red(acol, 3, w, tx)                        # t0
    red(acol, 7, w, ty)                        # t1
    red(acol, 11, w, tz)                       # t2
    nc.gpsimd.tensor_mul(wcb, w, cb)
    nc.gpsimd.tensor_mul(wca, w, ca)
    nc.gpsimd.tensor_mul(wsa, w, sa)
    nc.gpsimd.tensor_mul(wsasb, wsa, sb)
    nc.gpsimd.tensor_mul(wcasb, wca, sb)
    red(acol, 2, w, sb, scale=pscl)            # M02
    red(acol, 0, wcb, cc, scale=pscl)          # M00
    red(acol, 1, wcb, sc_, scale=nscl)         # M01
    red(acol, 4, wca, sc_, scale=pscl)         # M10 part 1
    red(acolp, 0, wsasb, cc, scale=pscl)       # M10 part 2
    red(acol, 5, wca, cc, scale=pscl)          # M11 part 1
    red(acolp, 1, wsasb, sc_, scale=nscl)      # M11 part 2
    red(acol, 6, wsa, cb, scale=nscl)          # M12
    red(acol, 8, wsa, sc_, scale=pscl)         # M20 part 1
    red(acolp, 2, wcasb, cc, scale=nscl)       # M20 part 2
    red(acol, 9, wsa, cc, scale=pscl)          # M21 part 1
    red(acolp, 3, wcasb, sc_, scale=pscl)      # M21 part 2
    red(acol, 10, wca, cb, scale=pscl)         # M22

    # merge two-part sums: acol cols {4,5,8,9} += acolp cols {0,1,2,3}
    am = acol[:].rearrange("p (i j) -> p i j", j=4)
    nc.vector.tensor_tensor(am[:, 1:3, 0:2], am[:, 1:3, 0:2],
                            acolp[:].rearrange("p (i j) -> p i j", j=2),
                            ALU.add)

    acolh = wk.tile([PT, 32], F16, tag="acolh")
    nc.vector.tensor_copy(acolh[:, 0:12], acol[:])

    at = wk.tile([108, PT], F16, tag="at")
    for b in range(4):
        nc.vector.transpose(at[0:32, 32 * b:32 * b + 32],
                            acolh[32 * b:32 * b + 32, 0:32])
    # replicate to the other 3 row-group bases; GPSIMD+DVE keep the busy
    # ACT engine out of this small matmul-ready chain
    nc.vector.tensor_copy(at[32:44, :], at[0:12, :])
    nc.gpsimd.tensor_copy(at[64:76, :], at[0:12, :])
    nc.gpsimd.tensor_copy(at[96:108, :], at[0:12, :])
    return at


def _phase_mm(nc, t, pools, at, vt_rep, out_d):
    io, wk, scr, pp = pools
    out_t = io.tile([PT, V3], F16, tag="out")
    for bk in range(NBLK):
        lo, hi = _blk_cols(bk)
        ps = pp.tile([PT, BLKW], F32, tag="ps")
        for j in range((hi - lo + 511) // 512):
            clo = lo + 512 * j
            chi = min(clo + 512, V3)
            g = (clo // 512) % 4
            nc.tensor.matmul(ps[:, 512 * j:512 * j + (chi - clo)],
                             at[32 * g:32 * g + 12, :],
                             vt_rep[32 * g:32 * g + 12, clo:chi],
                             start=True, stop=True, tile_position=(32 * g, 0))
        if bk in DVE_BLOCKS:
            nc.vector.tensor_copy(out_t[:, lo:hi], ps[:, 0:hi - lo])
            nc.sync.dma_start(out_d[t * PT:(t + 1) * PT, lo:hi],
                              out_t[:, lo:hi])
        else:
            nc.scalar.copy(out_t[:, lo:hi], ps[:, 0:hi - lo])
            nc.sync.dma_start(out_d[t * PT:(t + 1) * PT, lo:hi],
                              out_t[:, lo:hi])


def build(loop_iters: int = 0, sim_safe: bool = False,
          bench_internal_out: bool = False, unroll: int = 0,
          barrier_between: bool = False, loop_unroll: int = 1):
    """Build + compile the per-core program. loop_iters=0 -> straight-line
    single pass (grading); loop_iters=N -> For_i loop whose body runs
    loop_unroll back-to-back passes (the passes pipeline across engines;
    For_i's all-engine barrier only fires once per iteration) for
    wall-clock timing. sim_safe accepted for API compat (unused)."""
    nc = bacc.Bacc("TRN2", target_bir_lowering=False, debug=False)
    vt_d = nc.dram_tensor("vt", [48, V3], F16, kind="ExternalInput")
    inp_d = nc.dram_tensor("inp", [SLOTS, INP_W], F16, kind="ExternalInput")
    if bench_internal_out:
        # timing builds write to internal DRAM (identical HBM traffic) and
        # expose only a tiny dummy output, so host<->device transfer noise
        # doesn't pollute wall-clock differencing.
        out_d = nc.dram_tensor("outbuf", [SLOTS, V3], F16)
        dummy_d = nc.dram_tensor("out", [1, 16], F32, kind="ExternalOutput")
    else:
        out_d = nc.dram_tensor("out", [SLOTS, V3], F16, kind="ExternalOutput")
        dummy_d = None

    with tile.TileContext(nc) as tc:
        with (
            tc.tile_pool(name="const", bufs=1) as cpool,
            tc.tile_pool(name="io", bufs=3) as io,
            tc.tile_pool(name="wk", bufs=3) as wk,
            tc.tile_pool(name="scr", bufs=6) as scr,
            tc.tile_pool(name="psum", bufs=2, space="PSUM") as pp,
        ):
            # interleaved verts_h^T table replicated at the 4 row-group bases;
            # loaded via the GPSIMD SWDGE ring so it never queues ahead of the
            # latency-critical input loads (first use is the first matmul)
            vt_rep = cpool.tile([108, V3], F16)
            for g in range(4):
                nc.gpsimd.dma_start(vt_rep[32 * g:32 * g + 12, :],
                                    vt_d[12 * g:12 * g + 12, :])
            pools = (io, wk, scr, pp)

            def passes():
                inps, pa, ats = [], [], []
                for t in range(NT):
                    inp_t = io.tile([PT, INP_W], F16, tag="inp")
                    nc.sync.dma_start(inp_t[:], inp_d[t * PT:(t + 1) * PT, :])
                    inps.append(inp_t)
                for t in range(NT):
                    pa.append(_prep_a(nc, pools, inps[t]))
                for t in range(NT):
                    ats.append(_prep_b(nc, pools, inps[t], *pa[t]))
                for t in range(NT):
                    _phase_mm(nc, t, pools, ats[t], vt_rep, out_d)

            if loop_iters:
                with tc.For_i(0, loop_iters, 1):
                    for _ in range(loop_unroll):
                        passes()
            elif unroll:
                # straight-line repetition for TimelineSim measurement
                # (For_i needs register state the sim lacks); barriers
                # between passes mimic For_i's per-iteration
                # InstAllEngineBarrier
                for i in range(unroll):
                    if i and barrier_between:
                        nc.all_engine_barrier()
                    passes()
            else:
                passes()
            if dummy_d is not None:
                dtile = cpool.tile([1, 16], F32)
                nc.vector.memset(dtile[:], 1.0)
                nc.sync.dma_start(dummy_d[:], dtile[:])

    nc.compile()
    return nc


def _shard_inputs(verts, scales, transforms, prototype_weights):
    verts = np.ascontiguousarray(verts, dtype=np.float32)
    vh = np.concatenate([verts.T, np.ones((1, V), np.float32)],
                        axis=0).astype(np.float16)          # [4, V]
    vt12 = np.zeros((12, V3), np.float16)
    for i in range(3):
        vt12[4 * i:4 * i + 4, i::3] = vh
    vt48 = np.ascontiguousarray(np.vstack([vt12] * 4))      # [48, 7686]

    tr = transforms.reshape(B * S, P * 6).astype(np.float16)
    w = prototype_weights.reshape(B * S, P).astype(np.float16)
    sc = scales.reshape(B * S, 1).astype(np.float16)
    packed = np.concatenate([tr, w, sc], axis=1)            # [2048, 449]

    in_maps = []
    for k in range(NCORES):
        sl = slice(k * SLOTS, (k + 1) * SLOTS)
        in_maps.append({"vt": vt48, "inp": np.ascontiguousarray(packed[sl])})
    return in_maps


_cached_nc = None


def kernel(verts, scales, transforms, prototype_weights):
    global _cached_nc
    verts = np.asarray(verts, dtype=np.float32)
    scales = np.asarray(scales, dtype=np.float32)
    transforms = np.asarray(transforms, dtype=np.float32)
    prototype_weights = np.asarray(prototype_weights, dtype=np.float32)
    if _cached_nc is None:
        _cached_nc = build(loop_iters=0)
    in_maps = _shard_inputs(verts, scales, transforms, prototype_weights)
    res = run_bass_kernel_spmd(_cached_nc, in_maps, core_ids=list(range(NCORES)))
    parts = [np.asarray(res.results[k]["out"]).astype(np.float32)
             .reshape(SLOTS, V, 3) for k in range(NCORES)]
    return np.concatenate(parts, axis=0)
